# revision 8
# baseline (speedup 1.0000x reference)
"""Distributed causal multi-head attention for one TRN2 chip (8 NeuronCores).

Sharding: batch (2) x head-groups (4 heads/core) -> 8 cores.
Core c handles batch c//4, heads [ (c%4)*4 , (c%4)*4+4 ).
Per core: QKV projections for its 4 heads, flash-style causal attention
with scores kept transposed (S^T = K @ Q^T) so the PV product needs no
transposes; V is augmented with a ones column so the softmax denominators
fall out of the same matmul (row 64 of each head's O^T psum).  Then an
AllGather of the attention output (pre-Wo, 4-core group = one batch) and
a column-sliced output projection.  Host assembles the 8 column/batch
shards.  Compute dtype bf16 (PSUM accumulation fp32), softmax in fp32.

Scheduling: the attention loop keeps the in-order PE queue saturated by
interleaving KT/V/Q projections for later chunks and the AllGather-gated
output projections as queued work items.  j-tiles are processed in pairs
sharing one 4-bank psum tile so off-diagonal exp runs as a single
[128,2048] activation (halves ACT instruction overhead).  Output
projections are drained lazily (a 2-item reservoir is held back) so PE
work remains to fill every AllGather flight window, including the final
one — this keeps the HAM clock gate warm through the tail.
"""

import sys
from collections import deque

import numpy as np

sys.path.insert(0, "/opt/trn_rl_repo")

import concourse.bass as bass  # noqa: E402
import concourse.bacc as bacc  # noqa: E402
import concourse.tile as tile  # noqa: E402
import concourse.mybir as mybir  # noqa: E402

F32 = mybir.dt.float32
BF16 = mybir.dt.bfloat16
ActFn = mybir.ActivationFunctionType

P = 128          # partition dim
CHUNK = 512      # i-chunk (matmul moving free dim, one psum bank of fp32)
DH = 64          # head dim
HPC = 4          # heads per core
HS = HPC * DH    # 256 per-core inner slice
DHA = DH + 1     # augmented head dim (ones column for softmax sums)
INNER = 1024     # total inner dim (16 heads x 64)
N_CORES = 8
GROUPS = [[0, 1, 2, 3], [4, 5, 6, 7]]


def build_nc(seq=2048, dim=1024, n_cores=N_CORES, groups=GROUPS, compile=True):
    """Build the SPMD Bass graph (identical on all cores)."""
    nch = seq // CHUNK          # i-chunks
    jpc = CHUNK // P            # j-tiles per chunk (4)
    njt = seq // P              # j-tiles
    nk = dim // P               # feature k-tiles
    nko = INNER // P            # inner k-tiles for the output projection
    grp = len(groups[0])        # replica group size (4)

    nc = bacc.Bacc("TRN2", target_bir_lowering=False, debug=False,
                   enable_asserts=False, num_devices=n_cores)

    xT = nc.dram_tensor("xT", [dim, seq], BF16, kind="ExternalInput").ap()
    wq = nc.dram_tensor("wq", [dim, HS], BF16, kind="ExternalInput").ap()
    wk = nc.dram_tensor("wk", [dim, HS], BF16, kind="ExternalInput").ap()
    wv = nc.dram_tensor("wv", [dim, HS], BF16, kind="ExternalInput").ap()
    wo = nc.dram_tensor("wo", [INNER, HS], BF16, kind="ExternalInput").ap()
    mask_c = nc.dram_tensor("mask_c", [P, P], BF16, kind="ExternalInput").ap()
    outT = nc.dram_tensor("outT", [HS, seq], BF16, kind="ExternalOutput").ap()

    dmaq = [None]  # round-robin DMA issue over the 3 DMA-capable queues

    with tile.TileContext(nc) as tc:
        with tc.tile_pool(name="sb", bufs=1) as sb, \
             tc.tile_pool(name="ps", bufs=1, space="PSUM") as ps, \
             tc.tile_pool(name="dram", bufs=1, space="DRAM") as dram:

            dma_engines = [nc.sync, nc.scalar, nc.gpsimd]

            def dma(dst, src):
                i = dmaq[0] = (dmaq[0] or 0) + 1
                dma_engines[i % 3].dma_start(dst, src)

            # ---- load inputs (first-needed first; spread across queues;
            # xt split column-wise so chunk-0/1 slices land early) ----
            xt = [sb.tile([P, seq], BF16, tag=f"xt{k}", name=f"xt{k}")
                  for k in range(nk)]
            wq_sb = [sb.tile([P, HS], BF16, tag=f"wq{k}", name=f"wq{k}")
                     for k in range(nk)]
            wk_sb = [sb.tile([P, HS], BF16, tag=f"wk{k}", name=f"wk{k}")
                     for k in range(nk)]
            wv_sb = [sb.tile([P, HS], BF16, tag=f"wv{k}", name=f"wv{k}")
                     for k in range(nk)]
            wo_sb = [sb.tile([P, HS], BF16, tag=f"wo{k}", name=f"wo{k}")
                     for k in range(nko)]
            mask_sb = sb.tile([P, P], BF16, tag="mask", name="mask")

            half = seq // 2
            for k in range(nk):
                dma(wq_sb[k][:], wq[k * P:(k + 1) * P, :])
                dma(xt[k][:, 0:half], xT[k * P:(k + 1) * P, 0:half])
                dma(wk_sb[k][:], wk[k * P:(k + 1) * P, :])
            for k in range(nk):
                dma(wv_sb[k][:], wv[k * P:(k + 1) * P, :])
                dma(xt[k][:, half:seq], xT[k * P:(k + 1) * P, half:seq])
            for k in range(nko):
                dma(wo_sb[k][:], wo[k * P:(k + 1) * P, :])
            nc.gpsimd.dma_start(mask_sb[:], mask_c[:])

            # warm up the collectives firmware while QKV runs (tiny: the
            # entry barrier's length varies run to run, so a real-sized
            # warmup can land in the critical CC window)
            warm_in = dram.tile([P, 4], BF16, tag="warm_i", name="warm_i")
            warm_out = dram.tile([grp * P, 4], BF16,
                                 tag="warm_o", name="warm_o")
            nc.sync.dma_start(warm_in[:], xT[0:P, 0:4])
            nc.gpsimd.collective_compute(
                "AllGather", mybir.AluOpType.bypass, replica_groups=groups,
                ins=[warm_in.opt()], outs=[warm_out.opt()])

            # persistent QKV results
            qt_sb = [sb.tile([P, seq], BF16, tag=f"qt{p}", name=f"qt{p}")
                     for p in range(2)]
            kt_sb = [sb.tile([P, seq], BF16, tag=f"kt{p}", name=f"kt{p}")
                     for p in range(2)]
            v_sb = [sb.tile([P, HPC * DHA], BF16, tag=f"v{j}", name=f"v{j}")
                    for j in range(njt)]
            ot_sb = [sb.tile([P, seq], BF16, tag=f"ot{p}", name=f"ot{p}")
                     for p in range(2)]

            # ---- interleavable work items (each emits one psum group) ----
            def emit_kt(pair, ch):
                pt = ps.tile([P, CHUNK], F32, tag="misc",
                             name=f"ktps{pair}_{ch}", bufs=2)
                for k in range(nk):
                    nc.tensor.matmul(
                        pt[:], lhsT=wk_sb[k][:, pair * P:(pair + 1) * P],
                        rhs=xt[k][:, ch * CHUNK:(ch + 1) * CHUNK],
                        start=(k == 0), stop=(k == nk - 1))
                nc.vector.tensor_copy(
                    kt_sb[pair][:, ch * CHUNK:(ch + 1) * CHUNK], pt[:])

            def emit_v(jt):
                pt = ps.tile([P, HS], F32, tag="misc",
                             name=f"vps{jt}", bufs=2)
                for k in range(nk):
                    nc.tensor.matmul(
                        pt[:], lhsT=xt[k][:, jt * P:(jt + 1) * P],
                        rhs=wv_sb[k][:],
                        start=(k == 0), stop=(k == nk - 1))
                nc.vector.tensor_copy(
                    v_sb[jt].rearrange("p (h d) -> p h d", h=HPC)[:, :, 0:DH],
                    pt.rearrange("p (h d) -> p h d", h=HPC))
                nc.vector.memset(
                    v_sb[jt].rearrange("p (h d) -> p h d", h=HPC)[:, :, DH:DHA],
                    1.0)

            def emit_qt(pair, ch):
                pt = ps.tile([P, CHUNK], F32, tag="misc",
                             name=f"qps{pair}_{ch}", bufs=2)
                for k in range(nk):
                    nc.tensor.matmul(
                        pt[:],
                        lhsT=wq_sb[k][:, pair * P:(pair + 1) * P],
                        rhs=xt[k][:, ch * CHUNK:(ch + 1) * CHUNK],
                        start=(k == 0), stop=(k == nk - 1))
                nc.vector.tensor_copy(
                    qt_sb[pair][:, ch * CHUNK:(ch + 1) * CHUNK], pt[:])

            def emit_proj(ci, m, slices):
                # transposed output block: outT[m*128:(m+1)*128, chunk ci]
                # = Wo[:, m-slice].T @ attT[:, chunk] over gathered k-tiles.
                c0 = ci * CHUNK
                op_ps = ps.tile([P, CHUNK], F32, tag="misc",
                                name=f"op{ci}_{m}", bufs=2)
                for n, (k, ag_t, coff) in enumerate(slices):
                    nc.tensor.matmul(
                        op_ps[:],
                        lhsT=wo_sb[k][:, m * P:(m + 1) * P],
                        rhs=ag_t[:, coff:coff + CHUNK],
                        start=(n == 0), stop=(n == nko - 1))
                o_sb = sb.tile([P, CHUNK], BF16, tag="osb",
                               name=f"o{ci}_{m}", bufs=2)
                nc.vector.tensor_copy(o_sb[:], op_ps[:])
                dma(outT[m * P:(m + 1) * P, c0:c0 + CHUNK], o_sb[:])

            work_early = deque()   # KT/V/Q for future chunks (not gated)
            work_late = deque()    # output projections (gated on AllGather)
            reserve = [2]          # keep this many late items for AG windows

            def pop_work(late_ok, late_floor=None):
                floor = reserve[0] if late_floor is None else late_floor
                if work_early:
                    work_early.popleft()()
                    if len(work_early) > 4:
                        work_early.popleft()()
                elif late_ok and len(work_late) > floor:
                    work_late.popleft()()

            def emit_ag_full(ci, bounce_in):
                # one AllGather for both head pairs of chunk `ci` (256KB —
                # amortizes the ncfw floor; rank-major rows land so that
                # gathered row-block k*128 is exactly attT k-tile k)
                bounce_out = dram.tile([grp * 2 * P, CHUNK], BF16,
                                       tag="boutf", name=f"boutf{ci}", bufs=2)
                nc.gpsimd.collective_compute(
                    "AllGather", mybir.AluOpType.bypass,
                    replica_groups=groups,
                    ins=[bounce_in.opt()], outs=[bounce_out.opt()])
                tiles = {}
                for k in range(nko):
                    t = sb.tile([P, CHUNK], BF16, tag=f"ag{k}",
                                name=f"ag{ci}_{k}", bufs=2)
                    dma(t[:], bounce_out[k * P:(k + 1) * P, :])
                    tiles[k] = t
                return tiles

            def emit_ag_pair(ci, pair):
                # half AllGather (one head pair) of the chunk `ci` — fired
                # right after that pair's normalize, so pair A overlaps the
                # second attention pass and both stay small (cheap on CC).
                c0 = ci * CHUNK
                bounce_in = dram.tile([P, CHUNK], BF16, tag=f"binh{pair}",
                                      name=f"binh{ci}_{pair}", bufs=2)
                bounce_out = dram.tile([grp * P, CHUNK], BF16,
                                       tag=f"bouth{pair}",
                                       name=f"bouth{ci}_{pair}", bufs=2)
                nc.sync.dma_start(bounce_in[:], ot_sb[pair][:, c0:c0 + CHUNK])
                nc.gpsimd.collective_compute(
                    "AllGather", mybir.AluOpType.bypass,
                    replica_groups=groups,
                    ins=[bounce_in.opt()], outs=[bounce_out.opt()])
                tiles = {}
                for r in range(grp):
                    k = 2 * r + pair
                    t = sb.tile([P, CHUNK], BF16, tag=f"ag{k}",
                                name=f"ag{ci}_{k}", bufs=2)
                    dma_engines[r % 3].dma_start(
                        t[:], bounce_out[r * P:(r + 1) * P, :])
                    tiles[k] = t
                return tiles

            # ---- upfront projections: chunk-0/1 Q, chunk-0 K, chunk-0 V
            # (Q/K/V for later chunks are deferred into the work queue).
            for pair in range(2):
                emit_qt(pair, 0)
            for pair in range(2):
                emit_kt(pair, 0)
            for jt in range(jpc):
                emit_v(jt)
            for pair in range(2):
                if nch > 1:
                    emit_qt(pair, 1)
            for ch in range(2, nch):
                for pair in range(2):
                    work_early.append(
                        lambda pair=pair, ch=ch: emit_qt(pair, ch))

            # ---- attention chunks ----
            last_parts = {}
            for ci in range(nch):
                jt_end = jpc * (ci + 1)
                c0 = ci * CHUNK
                last = ci == nch - 1

                if ci + 1 < nch:
                    for pair in range(2):
                        work_early.append(
                            lambda pair=pair, ch=ci + 1: emit_kt(pair, ch))
                    for jt in range(jpc * (ci + 1), jpc * (ci + 2)):
                        work_early.append(lambda jt=jt: emit_v(jt))

                binf = None if last else dram.tile(
                    [2 * P, CHUNK], BF16, tag="binf", name=f"binf{ci}",
                    bufs=2)

                for hpass in range(2):
                    # heads 2*hpass, 2*hpass+1  (== head pair `hpass`)
                    ot_ps = [ps.tile([DHA, CHUNK], F32, tag=f"ot{h2}",
                                     name=f"ot{ci}_{hpass}_{h2}", bufs=1)
                             for h2 in range(2)]
                    # j-tiles processed in pairs sharing one 4-bank psum so
                    # the off-diagonal exp is a single wide activation
                    for jp in range(jt_end // 2):
                        jts = (2 * jp, 2 * jp + 1)
                        s4 = ps.tile([P, 4 * CHUNK], F32, tag="s4",
                                     name=f"s{ci}_{hpass}_{jp}", bufs=1)
                        es = sb.tile([P, 4 * CHUNK], BF16, tag="es",
                                     name=f"es{ci}_{hpass}_{jp}", bufs=2)
                        rels = []
                        for q, jt in enumerate(jts):
                            rel = max(0, (jt - jpc * ci)) * P
                            rels.append(rel)
                            for h2 in range(2):
                                # S^T tile = K_h @ Q_h^T (row-tiled, K=64)
                                nc.tensor.matmul(
                                    s4[:, (2 * q + h2) * CHUNK + rel:
                                       (2 * q + h2 + 1) * CHUNK],
                                    lhsT=kt_sb[hpass][h2 * DH:(h2 + 1) * DH,
                                                      jt * P:(jt + 1) * P],
                                    rhs=qt_sb[hpass][h2 * DH:(h2 + 1) * DH,
                                                     c0 + rel:c0 + CHUNK],
                                    start=True, stop=True,
                                    tile_position=(h2 * DH, 0))
                        diag = jts[0] >= jpc * ci
                        if not diag:
                            # one exp for both j-tiles and both heads
                            nc.scalar.activation(es[:], s4[:], ActFn.Exp)
                        else:
                            for q, jt in enumerate(jts):
                                rel = rels[q]
                                nc.scalar.activation(
                                    es.rearrange("p (t c) -> p t c",
                                                 t=4)[:, 2 * q:2 * q + 2,
                                                      rel:],
                                    s4.rearrange("p (t c) -> p t c",
                                                 t=4)[:, 2 * q:2 * q + 2,
                                                      rel:],
                                    ActFn.Exp)
                                # band mask on the diagonal block, both heads
                                nc.vector.tensor_mul(
                                    es.rearrange("p (t c) -> p t c",
                                                 t=4)[:, 2 * q:2 * q + 2,
                                                      rel:rel + P],
                                    es.rearrange("p (t c) -> p t c",
                                                 t=4)[:, 2 * q:2 * q + 2,
                                                      rel:rel + P],
                                    mask_sb.rearrange(
                                        "p (o c) -> p o c",
                                        o=1).broadcast_to((P, 2, P)))
                        for q, jt in enumerate(jts):
                            rel = rels[q]
                            for h2 in range(2):
                                h = 2 * hpass + h2
                                # O^T(+sums) accumulation: V_aug^T @ expS^T
                                nc.tensor.matmul(
                                    ot_ps[h2][:, rel:CHUNK],
                                    lhsT=v_sb[jt][:, h * DHA:(h + 1) * DHA],
                                    rhs=es[:, (2 * q + h2) * CHUNK + rel:
                                           (2 * q + h2 + 1) * CHUNK],
                                    start=(jt == 0), stop=(jt == jt_end - 1))
                        pop_work(late_ok=(hpass == 1 or last),
                                 late_floor=1 if last else None)

                    # normalize: rcp of the sums row (both heads fused),
                    # gpsimd partition-broadcast, then one psum-read mul
                    # per head writes the normalized O^T to SBUF
                    sr2 = sb.tile([1, 2 * CHUNK], F32, tag="sr",
                                  name=f"sr{ci}_{hpass}", bufs=2)
                    rcp2 = sb.tile([1, 2 * CHUNK], F32, tag="rcp",
                                   name=f"rcp{ci}_{hpass}", bufs=2)
                    bc2 = sb.tile([DH, 2 * CHUNK], F32, tag="bc",
                                  name=f"bc{ci}_{hpass}", bufs=2)
                    for h2 in range(2):
                        nc.vector.tensor_copy(
                            sr2[:, h2 * CHUNK:(h2 + 1) * CHUNK],
                            ot_ps[h2][DH:DHA, :])
                    nc.vector.reciprocal_approx_fast(rcp2[:], sr2[:])
                    nc.gpsimd.partition_broadcast(bc2[:], rcp2[:],
                                                  channels=DH)
                    for h2 in range(2):
                        nc.vector.tensor_mul(
                            ot_sb[hpass][h2 * DH:(h2 + 1) * DH,
                                         c0:c0 + CHUNK],
                            ot_ps[h2][0:DH, :],
                            bc2[:, h2 * CHUNK:(h2 + 1) * CHUNK])

                    if last:
                        # this pair's half of the chunk goes out now
                        last_parts.update(emit_ag_pair(ci, hpass))
                    else:
                        nc.sync.dma_start(
                            binf[hpass * P:(hpass + 1) * P, :],
                            ot_sb[hpass][:, c0:c0 + CHUNK])

                if last:
                    agt = dict(last_parts)
                    last_parts = {}
                else:
                    agt = emit_ag_full(ci, binf)
                # even k-tiles (pair A) first: their AllGather lands earlier
                korder = [k for k in range(nko) if k % 2 == 0] + \
                         [k for k in range(nko) if k % 2 == 1]
                slices = [(k, agt[k], 0) for k in korder]
                for m in range(HS // P):
                    work_late.append(
                        lambda ci=ci, m=m, s=slices: emit_proj(ci, m, s))

            # tail: reservoir drains now — the reserved projections fill
            # the last AllGather's flight window before the gated ones run
            reserve[0] = 0
            while work_early or work_late:
                pop_work(late_ok=True)

    if compile:
        nc.compile()
    return nc


def make_in_maps(x, Wq, Wk, Wv, Wo, n_cores=N_CORES):
    import ml_dtypes
    bf16 = ml_dtypes.bfloat16
    scale = np.float32(DH ** -0.5)
    # band mask for the diagonal j-tile of S^T [j,i]: keep j <= i
    mask_b = np.triu(np.ones((P, P), np.float32)).astype(bf16)
    in_maps = []
    for c in range(n_cores):
        b, r = divmod(c, 4)
        hs = r * HS
        in_maps.append({
            "xT": np.ascontiguousarray(x[b].T).astype(bf16),
            "wq": (Wq[:, hs:hs + HS] * scale).astype(bf16),
            "wk": np.ascontiguousarray(Wk[:, hs:hs + HS]).astype(bf16),
            "wv": np.ascontiguousarray(Wv[:, hs:hs + HS]).astype(bf16),
            "wo": np.ascontiguousarray(Wo[:, hs:hs + HS]).astype(bf16),
            "mask_c": mask_b,
        })
    return in_maps


def assemble_out(results, B, seq, n_cores=N_CORES):
    out = np.empty((B, seq, INNER), np.float32)
    for c in range(n_cores):
        b, r = divmod(c, 4)
        out[b][:, r * HS:(r + 1) * HS] = results[c]["outT"].T.astype(np.float32)
    return out


_NC_CACHE = {}


def kernel(x, Wq, Wk, Wv, Wo):
    from concourse import bass_utils
    x = np.asarray(x, np.float32)
    B, seq, dim = x.shape
    key = (seq, dim)
    if key not in _NC_CACHE:
        _NC_CACHE[key] = build_nc(seq=seq, dim=dim)
    nc = _NC_CACHE[key]
    in_maps = make_in_maps(x, np.asarray(Wq, np.float32),
                           np.asarray(Wk, np.float32),
                           np.asarray(Wv, np.float32),
                           np.asarray(Wo, np.float32))
    res = bass_utils.run_bass_kernel_spmd(
        nc, in_maps, core_ids=list(range(N_CORES)))
    return assemble_out(res.results, B, seq)


# revision 16
# speedup vs baseline: 1.0947x; 1.0947x over previous
"""Distributed causal multi-head attention for one TRN2 chip (8 NeuronCores).

Sharding: batch (2) x head-groups (4 heads/core) -> 8 cores.
Core c handles batch c//4, heads [ (c%4)*4 , (c%4)*4+4 ).
Per core: QKV projections for its 4 heads, flash-style causal attention
with scores kept transposed (S^T = K @ Q^T) so the PV product needs no
transposes; V is augmented with a ones column so the softmax denominators
fall out of the same matmul (row 64 of each head's O^T psum).  Then an
AllGather of the attention output (pre-Wo, 4-core group = one batch) and
a column-sliced output projection.  Host assembles the 8 column/batch
shards.  Compute dtype bf16 (PSUM accumulation fp32), softmax in fp32.

Scheduling: the attention loop keeps the in-order PE queue saturated by
interleaving KT/V/Q projections for later chunks and the AllGather-gated
output projections as queued work items.  j-tiles are processed in pairs
sharing one 4-bank psum tile so off-diagonal exp runs as a single
[128,2048] activation (halves ACT instruction overhead).  Output
projections are drained lazily (a 2-item reservoir is held back) so PE
work remains to fill every AllGather flight window, including the final
one — this keeps the HAM clock gate warm through the tail.
"""

import sys
from collections import deque

import numpy as np

sys.path.insert(0, "/opt/trn_rl_repo")

import concourse.bass as bass  # noqa: E402
import concourse.bacc as bacc  # noqa: E402
import concourse.tile as tile  # noqa: E402
import concourse.mybir as mybir  # noqa: E402

F32 = mybir.dt.float32
BF16 = mybir.dt.bfloat16
ActFn = mybir.ActivationFunctionType

P = 128          # partition dim
CHUNK = 512      # i-chunk (matmul moving free dim, one psum bank of fp32)
DH = 64          # head dim
HPC = 4          # heads per core
HS = HPC * DH    # 256 per-core inner slice
DHA = DH + 1     # augmented head dim (ones column for softmax sums)
INNER = 1024     # total inner dim (16 heads x 64)
N_CORES = 8
GROUPS = [[0, 1, 2, 3], [4, 5, 6, 7]]


def build_nc(seq=2048, dim=1024, n_cores=N_CORES, groups=GROUPS, compile=True):
    """Build the SPMD Bass graph (identical on all cores)."""
    nch = seq // CHUNK          # i-chunks
    jpc = CHUNK // P            # j-tiles per chunk (4)
    njt = seq // P              # j-tiles
    nk = dim // P               # feature k-tiles
    nko = INNER // P            # inner k-tiles for the output projection
    grp = len(groups[0])        # replica group size (4)

    nc = bacc.Bacc("TRN2", target_bir_lowering=False, debug=False,
                   enable_asserts=False, num_devices=n_cores)

    xT = nc.dram_tensor("xT", [dim, seq], BF16, kind="ExternalInput").ap()
    wq = nc.dram_tensor("wq", [dim, HS], BF16, kind="ExternalInput").ap()
    wk = nc.dram_tensor("wk", [dim, HS], BF16, kind="ExternalInput").ap()
    wv = nc.dram_tensor("wv", [dim, HS], BF16, kind="ExternalInput").ap()
    wo = nc.dram_tensor("wo", [INNER, HS], BF16, kind="ExternalInput").ap()
    mask_c = nc.dram_tensor("mask_c", [P, P], BF16, kind="ExternalInput").ap()
    outT = nc.dram_tensor("outT", [HS, seq], BF16, kind="ExternalOutput").ap()

    dmaq = [None]  # round-robin DMA issue over the 3 DMA-capable queues

    with tile.TileContext(nc) as tc:
        with tc.tile_pool(name="sb", bufs=1) as sb, \
             tc.tile_pool(name="ps", bufs=1, space="PSUM") as ps, \
             tc.tile_pool(name="dram", bufs=1, space="DRAM") as dram:

            dma_engines = [nc.sync, nc.scalar, nc.gpsimd]

            def dma(dst, src):
                # round-robin over the 3 DMA queues — ONLY for ungated
                # transfers (sources already resident).  AllGather-gated
                # loads must stay on nc.sync: a gated DMA at the head of
                # the scalar/gpsimd queue blocks exp/broadcast behind it.
                i = dmaq[0] = (dmaq[0] or 0) + 1
                dma_engines[i % 3].dma_start(dst, src)

            # warm up the collectives firmware first thing (tiny: the entry
            # barrier's length varies run to run, so a real-sized warmup can
            # land in the critical CC window; triggering before any queued
            # input DMAs keeps the CC stream clear for chunk 0's AllGather)
            warm_in = dram.tile([P, 4], BF16, tag="warm_i", name="warm_i")
            warm_out = dram.tile([grp * P, 4], BF16,
                                 tag="warm_o", name="warm_o")
            nc.sync.dma_start(warm_in[:], xT[0:P, 0:4])
            nc.gpsimd.collective_compute(
                "AllGather", mybir.AluOpType.bypass, replica_groups=groups,
                ins=[warm_in.opt()], outs=[warm_out.opt()])

            # ---- load inputs.  Each tensor family lives in one wide SBUF
            # tile so a single 3D-AP DMA moves many k-tiles (DMA issue slots
            # are ~600ns each on the engine queue — batching matters).  xt
            # is split column-wise (chunk-0/1 halves first) and k-grouped
            # across the three DMA queues. ----
            xts = sb.tile([P, nk * seq], BF16, tag="xts", name="xts")
            xt = [xts[:, k * seq:(k + 1) * seq] for k in range(nk)]
            wqs = sb.tile([P, nk * HS], BF16, tag="wqs", name="wqs")
            wq_sb = [wqs[:, k * HS:(k + 1) * HS] for k in range(nk)]
            wks = sb.tile([P, nk * HS], BF16, tag="wks", name="wks")
            wk_sb = [wks[:, k * HS:(k + 1) * HS] for k in range(nk)]
            wvs = sb.tile([P, nk * HS], BF16, tag="wvs", name="wvs")
            wv_sb = [wvs[:, k * HS:(k + 1) * HS] for k in range(nk)]
            wos = sb.tile([P, nko * HS], BF16, tag="wos", name="wos")
            wo_sb = [wos[:, k * HS:(k + 1) * HS] for k in range(nko)]
            mask_sb = sb.tile([P, P], BF16, tag="mask", name="mask")

            def w3d(w, n):
                # DRAM [n*P, c] viewed as [P, n, c] (partition, k-tile, col)
                return w.rearrange("(n p) c -> p n c", p=P)

            def x3d(xcols, k0, k1):
                # xT[k0*P:k1*P, cols] viewed as [P, k, cols]
                return xT[k0 * P:k1 * P, xcols].rearrange(
                    "(n p) c -> p n c", p=P)

            def xsbuf(xcols, k0, k1):
                return xts.rearrange("p (n c) -> p n c", n=nk)[:, k0:k1, xcols]

            half = seq // 2
            h0, h1 = slice(0, half), slice(half, seq)
            nc.sync.dma_start(wqs.rearrange("p (n c) -> p n c", n=nk),
                              w3d(wq, nk))
            nc.scalar.dma_start(wks.rearrange("p (n c) -> p n c", n=nk),
                                w3d(wk, nk))
            kb = max(1, (nk + 2) // 3)
            kg = [(a, min(a + kb, nk)) for a in range(0, nk, kb)]
            for (k0, k1), eng in zip(kg, dma_engines):
                eng.dma_start(xsbuf(h0, k0, k1), x3d(h0, k0, k1))
            nc.gpsimd.dma_start(wvs.rearrange("p (n c) -> p n c", n=nk),
                                w3d(wv, nk))
            for (k0, k1), eng in zip(kg, dma_engines):
                eng.dma_start(xsbuf(h1, k0, k1), x3d(h1, k0, k1))
            nc.scalar.dma_start(wos.rearrange("p (n c) -> p n c", n=nko),
                                w3d(wo, nko))
            nc.gpsimd.dma_start(mask_sb[:], mask_c[:])

            # persistent QKV results
            qt_sb = [sb.tile([P, seq], BF16, tag=f"qt{p}", name=f"qt{p}")
                     for p in range(2)]
            kt_sb = [sb.tile([P, seq], BF16, tag=f"kt{p}", name=f"kt{p}")
                     for p in range(2)]
            v_sb = [sb.tile([P, HPC * DHA], BF16, tag=f"v{j}", name=f"v{j}")
                    for j in range(njt)]
            ot_sb = [sb.tile([P, seq], BF16, tag=f"ot{p}", name=f"ot{p}")
                     for p in range(2)]

            # ---- interleavable work items (each emits one psum group) ----
            def emit_kt(pair, ch):
                pt = ps.tile([P, CHUNK], F32, tag="misc",
                             name=f"ktps{pair}_{ch}", bufs=2)
                for k in range(nk):
                    nc.tensor.matmul(
                        pt[:], lhsT=wk_sb[k][:, pair * P:(pair + 1) * P],
                        rhs=xt[k][:, ch * CHUNK:(ch + 1) * CHUNK],
                        start=(k == 0), stop=(k == nk - 1))
                nc.vector.tensor_copy(
                    kt_sb[pair][:, ch * CHUNK:(ch + 1) * CHUNK], pt[:])

            def emit_v(jt):
                pt = ps.tile([P, HS], F32, tag="misc",
                             name=f"vps{jt}", bufs=2)
                for k in range(nk):
                    nc.tensor.matmul(
                        pt[:], lhsT=xt[k][:, jt * P:(jt + 1) * P],
                        rhs=wv_sb[k][:],
                        start=(k == 0), stop=(k == nk - 1))
                nc.vector.tensor_copy(
                    v_sb[jt].rearrange("p (h d) -> p h d", h=HPC)[:, :, 0:DH],
                    pt.rearrange("p (h d) -> p h d", h=HPC))
                nc.vector.memset(
                    v_sb[jt].rearrange("p (h d) -> p h d", h=HPC)[:, :, DH:DHA],
                    1.0)

            def emit_qt(pair, ch):
                pt = ps.tile([P, CHUNK], F32, tag="misc",
                             name=f"qps{pair}_{ch}", bufs=2)
                for k in range(nk):
                    nc.tensor.matmul(
                        pt[:],
                        lhsT=wq_sb[k][:, pair * P:(pair + 1) * P],
                        rhs=xt[k][:, ch * CHUNK:(ch + 1) * CHUNK],
                        start=(k == 0), stop=(k == nk - 1))
                nc.vector.tensor_copy(
                    qt_sb[pair][:, ch * CHUNK:(ch + 1) * CHUNK], pt[:])

            def emit_proj(ci, m, slices):
                # transposed output block: outT[m*128:(m+1)*128, chunk ci]
                # = Wo[:, m-slice].T @ attT[:, chunk] over gathered k-tiles.
                c0 = ci * CHUNK
                op_ps = ps.tile([P, CHUNK], F32, tag="misc",
                                name=f"op{ci}_{m}", bufs=2)
                for n, (k, ag_t, coff) in enumerate(slices):
                    nc.tensor.matmul(
                        op_ps[:],
                        lhsT=wo_sb[k][:, m * P:(m + 1) * P],
                        rhs=ag_t[:, coff:coff + CHUNK],
                        start=(n == 0), stop=(n == nko - 1))
                o_sb = sb.tile([P, CHUNK], BF16, tag="osb",
                               name=f"o{ci}_{m}", bufs=2)
                nc.vector.tensor_copy(o_sb[:], op_ps[:])
                nc.sync.dma_start(outT[m * P:(m + 1) * P, c0:c0 + CHUNK],
                                  o_sb[:])

            work_early = deque()   # KT/V/Q for future chunks (not gated)
            work_late = deque()    # output projections (gated on AllGather)
            reserve = [2]          # keep this many late items for AG windows

            def pop_work(late_ok, late_floor=None):
                floor = reserve[0] if late_floor is None else late_floor
                if work_early:
                    work_early.popleft()()
                    if len(work_early) > 4:
                        work_early.popleft()()
                elif late_ok and len(work_late) > floor:
                    work_late.popleft()()

            def emit_ag_full(ci, bounce_in):
                # one AllGather for both head pairs of chunk `ci` (256KB —
                # amortizes the ncfw floor; rank-major rows land so that
                # gathered row-block k*128 is exactly attT k-tile k)
                bounce_out = dram.tile([grp * 2 * P, CHUNK], BF16,
                                       tag="boutf", name=f"boutf{ci}", bufs=2)
                nc.gpsimd.collective_compute(
                    "AllGather", mybir.AluOpType.bypass,
                    replica_groups=groups,
                    ins=[bounce_in.opt()], outs=[bounce_out.opt()])
                tiles = {}
                for k in range(nko):
                    # gated loads stay on sync: on scalar/gpsimd they would
                    # head-of-line-block exp/broadcast until the AG lands
                    t = sb.tile([P, CHUNK], BF16, tag=f"ag{k}",
                                name=f"ag{ci}_{k}", bufs=2)
                    nc.sync.dma_start(t[:], bounce_out[k * P:(k + 1) * P, :])
                    tiles[k] = t
                return tiles

            def emit_ag_pair(ci, pair):
                # half AllGather (one head pair) of the chunk `ci` — fired
                # right after that pair's normalize, so pair A overlaps the
                # second attention pass and both stay small (cheap on CC).
                c0 = ci * CHUNK
                bounce_in = dram.tile([P, CHUNK], BF16, tag=f"binh{pair}",
                                      name=f"binh{ci}_{pair}", bufs=2)
                bounce_out = dram.tile([grp * P, CHUNK], BF16,
                                       tag=f"bouth{pair}",
                                       name=f"bouth{ci}_{pair}", bufs=2)
                nc.sync.dma_start(bounce_in[:], ot_sb[pair][:, c0:c0 + CHUNK])
                nc.gpsimd.collective_compute(
                    "AllGather", mybir.AluOpType.bypass,
                    replica_groups=groups,
                    ins=[bounce_in.opt()], outs=[bounce_out.opt()])
                tiles = {}
                for r in range(grp):
                    k = 2 * r + pair
                    t = sb.tile([P, CHUNK], BF16, tag=f"ag{k}",
                                name=f"ag{ci}_{k}", bufs=2)
                    # pair B fires after the last exp/broadcast, so its
                    # gated loads can spread over all three DMA queues;
                    # pair A's must not block scalar/gpsimd mid-pass-B
                    eng = dma_engines[r % 3] if pair == 1 else nc.sync
                    eng.dma_start(t[:], bounce_out[r * P:(r + 1) * P, :])
                    tiles[k] = t
                return tiles

            # ---- upfront projections: chunk-0/1 Q, chunk-0 K, chunk-0 V
            # (Q/K/V for later chunks are deferred into the work queue).
            for pair in range(2):
                emit_qt(pair, 0)
            for pair in range(2):
                emit_kt(pair, 0)
            for jt in range(jpc):
                emit_v(jt)
            for pair in range(2):
                if nch > 1:
                    emit_qt(pair, 1)
            for ch in range(2, nch):
                for pair in range(2):
                    work_early.append(
                        lambda pair=pair, ch=ch: emit_qt(pair, ch))

            # ---- attention chunks ----
            last_parts = {}
            for ci in range(nch):
                jt_end = jpc * (ci + 1)
                c0 = ci * CHUNK
                last = ci == nch - 1

                if ci + 1 < nch:
                    for pair in range(2):
                        work_early.append(
                            lambda pair=pair, ch=ci + 1: emit_kt(pair, ch))
                    for jt in range(jpc * (ci + 1), jpc * (ci + 2)):
                        work_early.append(lambda jt=jt: emit_v(jt))

                binf = None if last else dram.tile(
                    [2 * P, CHUNK], BF16, tag="binf", name=f"binf{ci}",
                    bufs=2)

                for hpass in range(2):
                    # heads 2*hpass, 2*hpass+1  (== head pair `hpass`)
                    ot_ps = [ps.tile([DHA, CHUNK], F32, tag=f"ot{h2}",
                                     name=f"ot{ci}_{hpass}_{h2}", bufs=1)
                             for h2 in range(2)]
                    # j-tiles processed in pairs sharing one 4-bank psum so
                    # the off-diagonal exp is a single wide activation
                    for jp in range(jt_end // 2):
                        jts = (2 * jp, 2 * jp + 1)
                        s4 = ps.tile([P, 4 * CHUNK], F32, tag="s4",
                                     name=f"s{ci}_{hpass}_{jp}", bufs=1)
                        es = sb.tile([P, 4 * CHUNK], BF16, tag="es",
                                     name=f"es{ci}_{hpass}_{jp}", bufs=2)
                        rels = []
                        for q, jt in enumerate(jts):
                            rel = max(0, (jt - jpc * ci)) * P
                            rels.append(rel)
                            for h2 in range(2):
                                # S^T tile = K_h @ Q_h^T (row-tiled, K=64)
                                nc.tensor.matmul(
                                    s4[:, (2 * q + h2) * CHUNK + rel:
                                       (2 * q + h2 + 1) * CHUNK],
                                    lhsT=kt_sb[hpass][h2 * DH:(h2 + 1) * DH,
                                                      jt * P:(jt + 1) * P],
                                    rhs=qt_sb[hpass][h2 * DH:(h2 + 1) * DH,
                                                     c0 + rel:c0 + CHUNK],
                                    start=True, stop=True,
                                    tile_position=(h2 * DH, 0))
                        diag = jts[0] >= jpc * ci
                        if not diag:
                            # one exp for both j-tiles and both heads
                            nc.scalar.activation(es[:], s4[:], ActFn.Exp)
                        else:
                            for q, jt in enumerate(jts):
                                rel = rels[q]
                                nc.scalar.activation(
                                    es.rearrange("p (t c) -> p t c",
                                                 t=4)[:, 2 * q:2 * q + 2,
                                                      rel:],
                                    s4.rearrange("p (t c) -> p t c",
                                                 t=4)[:, 2 * q:2 * q + 2,
                                                      rel:],
                                    ActFn.Exp)
                                # band mask on the diagonal block, both heads
                                nc.vector.tensor_mul(
                                    es.rearrange("p (t c) -> p t c",
                                                 t=4)[:, 2 * q:2 * q + 2,
                                                      rel:rel + P],
                                    es.rearrange("p (t c) -> p t c",
                                                 t=4)[:, 2 * q:2 * q + 2,
                                                      rel:rel + P],
                                    mask_sb.rearrange(
                                        "p (o c) -> p o c",
                                        o=1).broadcast_to((P, 2, P)))
                        for q, jt in enumerate(jts):
                            rel = rels[q]
                            for h2 in range(2):
                                h = 2 * hpass + h2
                                # O^T(+sums) accumulation: V_aug^T @ expS^T
                                nc.tensor.matmul(
                                    ot_ps[h2][:, rel:CHUNK],
                                    lhsT=v_sb[jt][:, h * DHA:(h + 1) * DHA],
                                    rhs=es[:, (2 * q + h2) * CHUNK + rel:
                                           (2 * q + h2 + 1) * CHUNK],
                                    start=(jt == 0), stop=(jt == jt_end - 1))
                        # reservoir policy: drain gated projections only
                        # when their AllGather is surely complete; keep 2
                        # items back for the final AllGather flight window
                        pop_work(late_ok=(hpass == 1 or ci >= 2),
                                 late_floor=3 if hpass == 0 else 2)

                    # normalize: rcp of the sums row (both heads fused),
                    # gpsimd partition-broadcast, then one psum-read mul
                    # per head writes the normalized O^T to SBUF
                    sr2 = sb.tile([1, 2 * CHUNK], F32, tag="sr",
                                  name=f"sr{ci}_{hpass}", bufs=2)
                    rcp2 = sb.tile([1, 2 * CHUNK], F32, tag="rcp",
                                   name=f"rcp{ci}_{hpass}", bufs=2)
                    bc2 = sb.tile([DH, 2 * CHUNK], F32, tag="bc",
                                  name=f"bc{ci}_{hpass}", bufs=2)
                    for h2 in range(2):
                        nc.vector.tensor_copy(
                            sr2[:, h2 * CHUNK:(h2 + 1) * CHUNK],
                            ot_ps[h2][DH:DHA, :])
                    nc.vector.reciprocal_approx_fast(rcp2[:], sr2[:])
                    nc.gpsimd.partition_broadcast(bc2[:], rcp2[:],
                                                  channels=DH)
                    for h2 in range(2):
                        nc.vector.tensor_mul(
                            ot_sb[hpass][h2 * DH:(h2 + 1) * DH,
                                         c0:c0 + CHUNK],
                            ot_ps[h2][0:DH, :],
                            bc2[:, h2 * CHUNK:(h2 + 1) * CHUNK])

                    if last:
                        # this pair's half of the chunk goes out now
                        last_parts.update(emit_ag_pair(ci, hpass))
                    else:
                        nc.sync.dma_start(
                            binf[hpass * P:(hpass + 1) * P, :],
                            ot_sb[hpass][:, c0:c0 + CHUNK])

                if last:
                    agt = dict(last_parts)
                    last_parts = {}
                else:
                    agt = emit_ag_full(ci, binf)
                # even k-tiles (pair A) first: their AllGather lands earlier
                korder = [k for k in range(nko) if k % 2 == 0] + \
                         [k for k in range(nko) if k % 2 == 1]
                slices = [(k, agt[k], 0) for k in korder]
                for m in range(HS // P):
                    work_late.append(
                        lambda ci=ci, m=m, s=slices: emit_proj(ci, m, s))

            # tail: reservoir drains now — the reserved projections fill
            # the last AllGather's flight window before the gated ones run
            reserve[0] = 0
            while work_early or work_late:
                pop_work(late_ok=True, late_floor=0)

    if compile:
        nc.compile()
    return nc


def make_in_maps(x, Wq, Wk, Wv, Wo, n_cores=N_CORES):
    import ml_dtypes
    bf16 = ml_dtypes.bfloat16
    scale = np.float32(DH ** -0.5)
    # band mask for the diagonal j-tile of S^T [j,i]: keep j <= i
    mask_b = np.triu(np.ones((P, P), np.float32)).astype(bf16)
    in_maps = []
    for c in range(n_cores):
        b, r = divmod(c, 4)
        hs = r * HS
        in_maps.append({
            "xT": np.ascontiguousarray(x[b].T).astype(bf16),
            "wq": (Wq[:, hs:hs + HS] * scale).astype(bf16),
            "wk": np.ascontiguousarray(Wk[:, hs:hs + HS]).astype(bf16),
            "wv": np.ascontiguousarray(Wv[:, hs:hs + HS]).astype(bf16),
            "wo": np.ascontiguousarray(Wo[:, hs:hs + HS]).astype(bf16),
            "mask_c": mask_b,
        })
    return in_maps


def assemble_out(results, B, seq, n_cores=N_CORES):
    out = np.empty((B, seq, INNER), np.float32)
    for c in range(n_cores):
        b, r = divmod(c, 4)
        out[b][:, r * HS:(r + 1) * HS] = results[c]["outT"].T.astype(np.float32)
    return out


_NC_CACHE = {}


def kernel(x, Wq, Wk, Wv, Wo):
    from concourse import bass_utils
    x = np.asarray(x, np.float32)
    B, seq, dim = x.shape
    key = (seq, dim)
    if key not in _NC_CACHE:
        _NC_CACHE[key] = build_nc(seq=seq, dim=dim)
    nc = _NC_CACHE[key]
    in_maps = make_in_maps(x, np.asarray(Wq, np.float32),
                           np.asarray(Wk, np.float32),
                           np.asarray(Wv, np.float32),
                           np.asarray(Wo, np.float32))
    res = bass_utils.run_bass_kernel_spmd(
        nc, in_maps, core_ids=list(range(N_CORES)))
    return assemble_out(res.results, B, seq)


# revision 20
# speedup vs baseline: 1.2571x; 1.1483x over previous
"""Distributed causal multi-head attention for one TRN2 chip (8 NeuronCores).

Sharding: batch (2) x head-groups (4 heads/core) -> 8 cores.
Core c handles batch c//4, heads [ (c%4)*4 , (c%4)*4+4 ).
Per core: QKV projections for its 4 heads, flash-style causal attention
with scores kept transposed (S^T = K @ Q^T) so the PV product needs no
transposes; V is augmented with a ones column so the softmax denominators
fall out of the same matmul (row 64 of each head's O^T psum).  Then an
AllGather of the attention output (pre-Wo, 4-core group = one batch) and
a column-sliced output projection.  Host assembles the 8 column/batch
shards.  Compute dtype bf16 (PSUM accumulation fp32), softmax in fp32.

Scheduling notes:
- All host inputs are repacked partition-major so every input tensor
  loads with a handful of large contiguous DMAs.
- The attention pipeline is ACT(exp)-rate-limited; KT/V/Q projections
  for later chunks and AllGather-gated output projections are queued
  work items drained into the PE's idle slots.
- AllGather-gated DMA loads ride the sync queue only: at the head of
  the scalar/gpsimd queue they would block exp/broadcast behind them.
- Projections are drained lazily (reservoir) so PE work remains to fill
  AllGather flight windows; the flight of the final half-chunk gather
  is covered by reserved projections plus the pair-A halves of the last
  chunk's projections (held psum accumulation), keeping the HAM clock
  gate warm through the tail.
"""

import sys
from collections import deque

import numpy as np

sys.path.insert(0, "/opt/trn_rl_repo")

import concourse.bass as bass  # noqa: E402
import concourse.bacc as bacc  # noqa: E402
import concourse.tile as tile  # noqa: E402
import concourse.mybir as mybir  # noqa: E402

F32 = mybir.dt.float32
BF16 = mybir.dt.bfloat16
ActFn = mybir.ActivationFunctionType

P = 128          # partition dim
CHUNK = 512      # i-chunk (matmul moving free dim, one psum bank of fp32)
DH = 64          # head dim
HPC = 4          # heads per core
HS = HPC * DH    # 256 per-core inner slice
DHA = DH + 1     # augmented head dim (ones column for softmax sums)
INNER = 1024     # total inner dim (16 heads x 64)
N_CORES = 8
GROUPS = [[0, 1, 2, 3], [4, 5, 6, 7]]


def build_nc(seq=2048, dim=1024, n_cores=N_CORES, groups=GROUPS, compile=True):
    """Build the SPMD Bass graph (identical on all cores)."""
    nch = seq // CHUNK          # i-chunks
    jpc = CHUNK // P            # j-tiles per chunk (4)
    njt = seq // P              # j-tiles
    nk = dim // P               # feature k-tiles
    nko = INNER // P            # inner k-tiles for the output projection
    grp = len(groups[0])        # replica group size (4)

    nc = bacc.Bacc("TRN2", target_bir_lowering=False, debug=False,
                   enable_asserts=False, num_devices=n_cores)

    # all inputs are host-repacked partition-major: [P, ntiles*cols]
    xT = nc.dram_tensor("xT", [P, nk * seq], BF16, kind="ExternalInput").ap()
    wq = nc.dram_tensor("wq", [P, nk * HS], BF16, kind="ExternalInput").ap()
    wk = nc.dram_tensor("wk", [P, nk * HS], BF16, kind="ExternalInput").ap()
    wv = nc.dram_tensor("wv", [P, nk * HS], BF16, kind="ExternalInput").ap()
    wo = nc.dram_tensor("wo", [P, nko * HS], BF16, kind="ExternalInput").ap()
    mask_c = nc.dram_tensor("mask_c", [P, P], BF16, kind="ExternalInput").ap()
    outT = nc.dram_tensor("outT", [HS, seq], BF16, kind="ExternalOutput").ap()

    with tile.TileContext(nc) as tc:
        with tc.tile_pool(name="sb", bufs=1) as sb, \
             tc.tile_pool(name="ps", bufs=1, space="PSUM") as ps, \
             tc.tile_pool(name="dram", bufs=1, space="DRAM") as dram:

            dma_engines = [nc.sync, nc.scalar, nc.gpsimd]

            # warm up the collectives firmware first thing (tiny: the entry
            # barrier's length varies run to run; triggering before any
            # queued input DMAs keeps the CC stream clear for chunk 0's AG)
            warm_in = dram.tile([P, 4], BF16, tag="warm_i", name="warm_i")
            warm_out = dram.tile([grp * P, 4], BF16,
                                 tag="warm_o", name="warm_o")
            nc.sync.dma_start(warm_in[:], xT[0:P, 0:4])
            nc.gpsimd.collective_compute(
                "AllGather", mybir.AluOpType.bypass, replica_groups=groups,
                ins=[warm_in.opt()], outs=[warm_out.opt()])

            # ---- input loads: few, large, contiguous DMAs; xt halves
            # k-grouped across the three DMA queues so chunk-0 columns of
            # every k-tile land within the first few microseconds ----
            xts = sb.tile([P, nk * seq], BF16, tag="xts", name="xts")
            xt = [xts[:, k * seq:(k + 1) * seq] for k in range(nk)]
            wqs = sb.tile([P, nk * HS], BF16, tag="wqs", name="wqs")
            wq_sb = [wqs[:, k * HS:(k + 1) * HS] for k in range(nk)]
            wks = sb.tile([P, nk * HS], BF16, tag="wks", name="wks")
            wk_sb = [wks[:, k * HS:(k + 1) * HS] for k in range(nk)]
            wvs = sb.tile([P, nk * HS], BF16, tag="wvs", name="wvs")
            wv_sb = [wvs[:, k * HS:(k + 1) * HS] for k in range(nk)]
            wos = sb.tile([P, nko * HS], BF16, tag="wos", name="wos")
            wo_sb = [wos[:, k * HS:(k + 1) * HS] for k in range(nko)]
            mask_sb = sb.tile([P, P], BF16, tag="mask", name="mask")

            def kslice(t, k0, k1, cols):
                return t.rearrange("p (n c) -> p n c", n=nk)[:, k0:k1, cols]

            half = seq // 2
            h0, h1 = slice(0, half), slice(half, seq)
            kb = max(1, (nk + 2) // 3)
            kg = [(a, min(a + kb, nk)) for a in range(0, nk, kb)]
            nc.sync.dma_start(wqs[:], wq[:])
            nc.scalar.dma_start(wks[:], wk[:])
            nc.gpsimd.dma_start(wvs[:], wv[:])
            for (k0, k1), eng in zip(kg, dma_engines):
                eng.dma_start(kslice(xts, k0, k1, h0), kslice(xT, k0, k1, h0))
            for (k0, k1), eng in zip(kg, dma_engines):
                eng.dma_start(kslice(xts, k0, k1, h1), kslice(xT, k0, k1, h1))
            nc.scalar.dma_start(wos[:], wo[:])
            nc.gpsimd.dma_start(mask_sb[:], mask_c[:])

            # persistent QKV results
            qt_sb = [sb.tile([P, seq], BF16, tag=f"qt{p}", name=f"qt{p}")
                     for p in range(2)]
            kt_sb = [sb.tile([P, seq], BF16, tag=f"kt{p}", name=f"kt{p}")
                     for p in range(2)]
            v_sb = [sb.tile([P, HPC * DHA], BF16, tag=f"v{j}", name=f"v{j}")
                    for j in range(njt)]
            ot_sb = [sb.tile([P, seq], BF16, tag=f"ot{p}", name=f"ot{p}")
                     for p in range(2)]

            # ---- interleavable work items (each emits one psum group) ----
            def emit_kt(pair, ch):
                pt = ps.tile([P, CHUNK], F32, tag="misc",
                             name=f"ktps{pair}_{ch}", bufs=2)
                for k in range(nk):
                    nc.tensor.matmul(
                        pt[:], lhsT=wk_sb[k][:, pair * P:(pair + 1) * P],
                        rhs=xt[k][:, ch * CHUNK:(ch + 1) * CHUNK],
                        start=(k == 0), stop=(k == nk - 1))
                nc.vector.tensor_copy(
                    kt_sb[pair][:, ch * CHUNK:(ch + 1) * CHUNK], pt[:])

            def emit_v(jt):
                pt = ps.tile([P, HS], F32, tag="misc",
                             name=f"vps{jt}", bufs=2)
                for k in range(nk):
                    nc.tensor.matmul(
                        pt[:], lhsT=xt[k][:, jt * P:(jt + 1) * P],
                        rhs=wv_sb[k][:],
                        start=(k == 0), stop=(k == nk - 1))
                nc.vector.tensor_copy(
                    v_sb[jt].rearrange("p (h d) -> p h d", h=HPC)[:, :, 0:DH],
                    pt.rearrange("p (h d) -> p h d", h=HPC))
                nc.vector.memset(
                    v_sb[jt].rearrange("p (h d) -> p h d", h=HPC)[:, :, DH:DHA],
                    1.0)

            def emit_qt(pair, ch):
                pt = ps.tile([P, CHUNK], F32, tag="misc",
                             name=f"qps{pair}_{ch}", bufs=2)
                for k in range(nk):
                    nc.tensor.matmul(
                        pt[:],
                        lhsT=wq_sb[k][:, pair * P:(pair + 1) * P],
                        rhs=xt[k][:, ch * CHUNK:(ch + 1) * CHUNK],
                        start=(k == 0), stop=(k == nk - 1))
                nc.vector.tensor_copy(
                    qt_sb[pair][:, ch * CHUNK:(ch + 1) * CHUNK], pt[:])

            def emit_proj(ci, m, slices, op_ps=None, evac=True):
                # transposed output block: outT[m*128:(m+1)*128, chunk ci]
                # = Wo[:, m-slice].T @ attT[:, chunk] over the k-tiles in
                # `slices` (a partial pass keeps op_ps alive)
                c0 = ci * CHUNK
                first = op_ps is None
                if first:
                    op_ps = ps.tile([P, CHUNK], F32, tag="misc",
                                    name=f"op{ci}_{m}", bufs=2)
                for n, (k, ag_t, coff) in enumerate(slices):
                    nc.tensor.matmul(
                        op_ps[:],
                        lhsT=wo_sb[k][:, m * P:(m + 1) * P],
                        rhs=ag_t[:, coff:coff + CHUNK],
                        start=(first and n == 0),
                        stop=(evac and n == len(slices) - 1))
                if not evac:
                    return op_ps
                o_sb = sb.tile([P, CHUNK], BF16, tag="osb",
                               name=f"o{ci}_{m}", bufs=2)
                nc.vector.tensor_copy(o_sb[:], op_ps[:])
                nc.sync.dma_start(outT[m * P:(m + 1) * P, c0:c0 + CHUNK],
                                  o_sb[:])
                return None

            work_early = deque()   # KT/V/Q for future chunks (not gated)
            work_late = deque()    # output projections (gated on AllGather)

            def pop_work(late_ok, late_floor):
                if work_early:
                    work_early.popleft()()
                    if len(work_early) > 4:
                        work_early.popleft()()
                elif late_ok and len(work_late) > late_floor:
                    work_late.popleft()()

            def emit_ag_full(ci, bounce_in):
                # one AllGather for both head pairs of chunk `ci` (256KB —
                # amortizes the ncfw floor; rank-major rows land so that
                # gathered row-block k*128 is exactly attT k-tile k)
                bounce_out = dram.tile([grp * 2 * P, CHUNK], BF16,
                                       tag="boutf", name=f"boutf{ci}", bufs=2)
                nc.gpsimd.collective_compute(
                    "AllGather", mybir.AluOpType.bypass,
                    replica_groups=groups,
                    ins=[bounce_in.opt()], outs=[bounce_out.opt()])
                tiles = {}
                for k in range(nko):
                    # gated loads stay on sync: on scalar/gpsimd they would
                    # head-of-line-block exp/broadcast until the AG lands
                    t = sb.tile([P, CHUNK], BF16, tag=f"ag{k}",
                                name=f"ag{ci}_{k}", bufs=2)
                    nc.sync.dma_start(t[:], bounce_out[k * P:(k + 1) * P, :])
                    tiles[k] = t
                return tiles

            def emit_ag_pair(ci, pair):
                # half AllGather (one head pair) of the chunk `ci` — fired
                # right after that pair's normalize, so pair A overlaps the
                # second attention pass and both stay small (cheap on CC).
                c0 = ci * CHUNK
                bounce_in = dram.tile([P, CHUNK], BF16, tag=f"binh{pair}",
                                      name=f"binh{ci}_{pair}", bufs=2)
                bounce_out = dram.tile([grp * P, CHUNK], BF16,
                                       tag=f"bouth{pair}",
                                       name=f"bouth{ci}_{pair}", bufs=2)
                nc.sync.dma_start(bounce_in[:], ot_sb[pair][:, c0:c0 + CHUNK])
                nc.gpsimd.collective_compute(
                    "AllGather", mybir.AluOpType.bypass,
                    replica_groups=groups,
                    ins=[bounce_in.opt()], outs=[bounce_out.opt()])
                tiles = {}
                for r in range(grp):
                    k = 2 * r + pair
                    t = sb.tile([P, CHUNK], BF16, tag=f"ag{k}",
                                name=f"ag{ci}_{k}", bufs=2)
                    # pair B fires after the last exp/broadcast, so its
                    # gated loads can spread over all three DMA queues;
                    # pair A's must not block scalar/gpsimd mid-pass-B
                    eng = dma_engines[r % 3] if pair == 1 else nc.sync
                    eng.dma_start(t[:], bounce_out[r * P:(r + 1) * P, :])
                    tiles[k] = t
                return tiles

            # ---- upfront projections: chunk-0/1 Q, chunk-0 K, chunk-0 V
            # (Q/K/V for later chunks are deferred into the work queue).
            for pair in range(2):
                emit_qt(pair, 0)
            for pair in range(2):
                emit_kt(pair, 0)
            for jt in range(jpc):
                emit_v(jt)
            if nch > 1:
                for pair in range(2):
                    emit_qt(pair, 1)
            for ch in range(2, nch):
                for pair in range(2):
                    work_early.append(
                        lambda pair=pair, ch=ch: emit_qt(pair, ch))

            # ---- attention chunks ----
            last_parts = {}
            for ci in range(nch):
                jt_end = jpc * (ci + 1)
                c0 = ci * CHUNK
                last = ci == nch - 1

                if ci + 1 < nch:
                    for pair in range(2):
                        work_early.append(
                            lambda pair=pair, ch=ci + 1: emit_kt(pair, ch))
                    for jt in range(jpc * (ci + 1), jpc * (ci + 2)):
                        work_early.append(lambda jt=jt: emit_v(jt))

                binf = None if last else dram.tile(
                    [2 * P, CHUNK], BF16, tag="binf", name=f"binf{ci}",
                    bufs=2)

                for hpass in range(2):
                    # heads 2*hpass, 2*hpass+1  (== head pair `hpass`)
                    ot_ps = [ps.tile([DHA, CHUNK], F32, tag=f"ot{h2}",
                                     name=f"ot{ci}_{hpass}_{h2}", bufs=1)
                             for h2 in range(2)]
                    for jt in range(jt_end):
                        rel = max(0, (jt - jpc * ci)) * P
                        diag = jt >= jpc * ci

                        s2 = ps.tile([P, 2 * CHUNK], F32, tag="s2",
                                     name=f"s{ci}_{hpass}_{jt}", bufs=2)
                        es = sb.tile([P, 2 * CHUNK], BF16, tag="es",
                                     name=f"es{ci}_{hpass}_{jt}", bufs=3)

                        for h2 in range(2):
                            # S^T tile = K_h @ Q_h^T (row-tiled, K=64)
                            nc.tensor.matmul(
                                s2[:, h2 * CHUNK + rel:(h2 + 1) * CHUNK],
                                lhsT=kt_sb[hpass][h2 * DH:(h2 + 1) * DH,
                                                  jt * P:(jt + 1) * P],
                                rhs=qt_sb[hpass][h2 * DH:(h2 + 1) * DH,
                                                 c0 + rel:c0 + CHUNK],
                                start=True, stop=True,
                                tile_position=(h2 * DH, 0))
                        # one exp for both heads (both psum banks)
                        nc.scalar.activation(
                            es.rearrange("p (t c) -> p t c", t=2)[:, :, rel:],
                            s2.rearrange("p (t c) -> p t c", t=2)[:, :, rel:],
                            ActFn.Exp)
                        if diag:
                            # band mask on the diagonal block, both heads
                            nc.vector.tensor_mul(
                                es.rearrange("p (t c) -> p t c",
                                             t=2)[:, :, rel:rel + P],
                                es.rearrange("p (t c) -> p t c",
                                             t=2)[:, :, rel:rel + P],
                                mask_sb.rearrange(
                                    "p (o c) -> p o c",
                                    o=1).broadcast_to((P, 2, P)))
                        for h2 in range(2):
                            h = 2 * hpass + h2
                            # O^T(+sums) accumulation: V_aug^T @ expS^T
                            nc.tensor.matmul(
                                ot_ps[h2][:, rel:CHUNK],
                                lhsT=v_sb[jt][:, h * DHA:(h + 1) * DHA],
                                rhs=es[:, h2 * CHUNK + rel:(h2 + 1) * CHUNK],
                                start=(jt == 0), stop=(jt == jt_end - 1))
                        # reservoir: drain gated projections only when their
                        # AllGather is surely complete (pass B for the
                        # previous-previous chunk); keep 2 back for the tail
                        pop_work(late_ok=(hpass == 1 or last),
                                 late_floor=2 if (hpass == 1 or last) else 4)

                    # normalize: rcp of the sums row (both heads fused),
                    # partition-broadcast, then one psum-read mul per head
                    # writes the normalized O^T to SBUF.  The broadcast
                    # rides gpsimd except on the final pass, where a PE
                    # ones-matmul keeps gpsimd clear for the AG trigger.
                    sr2 = sb.tile([1, 2 * CHUNK], F32, tag="sr",
                                  name=f"sr{ci}_{hpass}", bufs=2)
                    rcp2 = sb.tile([1, 2 * CHUNK], F32, tag="rcp",
                                   name=f"rcp{ci}_{hpass}", bufs=2)
                    nc.scalar.activation(sr2[:, 0:CHUNK], ot_ps[0][DH:DHA, :],
                                         ActFn.Copy)
                    nc.vector.tensor_copy(sr2[:, CHUNK:2 * CHUNK],
                                          ot_ps[1][DH:DHA, :])
                    nc.vector.reciprocal_approx_fast(rcp2[:], sr2[:])
                    bc2 = sb.tile([DH, 2 * CHUNK], F32, tag="bc",
                                  name=f"bc{ci}_{hpass}", bufs=2)
                    nc.gpsimd.partition_broadcast(bc2[:], rcp2[:],
                                                  channels=DH)
                    for h2 in range(2):
                        nc.vector.tensor_mul(
                            ot_sb[hpass][h2 * DH:(h2 + 1) * DH,
                                         c0:c0 + CHUNK],
                            ot_ps[h2][0:DH, :],
                            bc2[:, h2 * CHUNK:(h2 + 1) * CHUNK])

                    if last:
                        # this pair's half of the chunk goes out now
                        last_parts.update(emit_ag_pair(ci, hpass))
                    else:
                        nc.sync.dma_start(
                            binf[hpass * P:(hpass + 1) * P, :],
                            ot_sb[hpass][:, c0:c0 + CHUNK])

                if last:
                    agt = dict(last_parts)
                    last_parts = {}
                else:
                    agt = emit_ag_full(ci, binf)
                evens = [(k, agt[k], 0) for k in range(0, nko, 2)]
                odds = [(k, agt[k], 0) for k in range(1, nko, 2)]
                nm = HS // P
                if not last:
                    for m in range(nm):
                        work_late.append(
                            lambda ci=ci, m=m, s=evens + odds:
                            emit_proj(ci, m, s))
                else:
                    # split each output block's projection: the even k-tiles
                    # (pair-A AllGather, long landed) fill the pair-B AG's
                    # flight window; psum held across the split (no other
                    # psum users remain at the tail)
                    op_tiles = {}

                    def proj_ev(ci, m):
                        op_tiles[m] = emit_proj(ci, m, evens, evac=False)

                    def proj_od(ci, m):
                        emit_proj(ci, m, odds, op_ps=op_tiles.pop(m))

                    for m in range(nm):
                        work_late.append(
                            lambda ci=ci, m=m: proj_ev(ci, m))
                    for m in range(nm):
                        work_late.append(
                            lambda ci=ci, m=m: proj_od(ci, m))

            # tail: reservoir drains now — the reserved projections and the
            # pair-A halves fill the final AllGather's flight window before
            # the gated pair-B halves run
            while work_early or work_late:
                pop_work(late_ok=True, late_floor=0)

    if compile:
        nc.compile()
    return nc


def _pack(a, p=P):
    # [n*p, c] -> [p, n*c] partition-major repack
    n = a.shape[0] // p
    return np.ascontiguousarray(
        a.reshape(n, p, a.shape[1]).transpose(1, 0, 2).reshape(p, -1))


def make_in_maps(x, Wq, Wk, Wv, Wo, n_cores=N_CORES):
    import ml_dtypes
    bf16 = ml_dtypes.bfloat16
    scale = np.float32(DH ** -0.5)
    # band mask for the diagonal j-tile of S^T [j,i]: keep j <= i
    mask_b = np.triu(np.ones((P, P), np.float32)).astype(bf16)
    in_maps = []
    for c in range(n_cores):
        b, r = divmod(c, 4)
        hs = r * HS
        in_maps.append({
            "xT": _pack(np.ascontiguousarray(x[b].T)).astype(bf16),
            "wq": _pack(Wq[:, hs:hs + HS] * scale).astype(bf16),
            "wk": _pack(Wk[:, hs:hs + HS]).astype(bf16),
            "wv": _pack(Wv[:, hs:hs + HS]).astype(bf16),
            "wo": _pack(Wo[:, hs:hs + HS]).astype(bf16),
            "mask_c": mask_b,
        })
    return in_maps


def assemble_out(results, B, seq, n_cores=N_CORES):
    out = np.empty((B, seq, INNER), np.float32)
    for c in range(n_cores):
        b, r = divmod(c, 4)
        out[b][:, r * HS:(r + 1) * HS] = results[c]["outT"].T.astype(np.float32)
    return out


_NC_CACHE = {}


def kernel(x, Wq, Wk, Wv, Wo):
    from concourse import bass_utils
    x = np.asarray(x, np.float32)
    B, seq, dim = x.shape
    key = (seq, dim)
    if key not in _NC_CACHE:
        _NC_CACHE[key] = build_nc(seq=seq, dim=dim)
    nc = _NC_CACHE[key]
    in_maps = make_in_maps(x, np.asarray(Wq, np.float32),
                           np.asarray(Wk, np.float32),
                           np.asarray(Wv, np.float32),
                           np.asarray(Wo, np.float32))
    res = bass_utils.run_bass_kernel_spmd(
        nc, in_maps, core_ids=list(range(N_CORES)))
    return assemble_out(res.results, B, seq)


# revision 22
# speedup vs baseline: 1.3575x; 1.0799x over previous
"""Distributed causal multi-head attention for one TRN2 chip (8 NeuronCores).

Sharding: batch (2) x head-groups (4 heads/core) -> 8 cores.
Core c handles batch c//4, heads [ (c%4)*4 , (c%4)*4+4 ).
Per core: QKV projections for its 4 heads, flash-style causal attention
with scores kept transposed (S^T = K @ Q^T) so the PV product needs no
transposes; V is augmented with a ones column so the softmax denominators
fall out of the same matmul (row 64 of each head's O^T psum).  Then an
AllGather of the attention output (pre-Wo, 4-core group = one batch) and
a column-sliced output projection.  Host assembles the 8 column/batch
shards.  Compute dtype bf16 (PSUM accumulation fp32), softmax in fp32.

Scheduling notes:
- All host inputs are repacked partition-major so every input tensor
  loads with a handful of large contiguous DMAs.
- The attention pipeline is ACT(exp)-rate-limited; KT/V/Q projections
  for later chunks and AllGather-gated output projections are queued
  work items drained into the PE's idle slots.
- AllGather-gated DMA loads ride the sync queue only: at the head of
  the scalar/gpsimd queue they would block exp/broadcast behind them.
- Projections are drained lazily (reservoir) so PE work remains to fill
  AllGather flight windows; the flight of the final half-chunk gather
  is covered by reserved projections plus the pair-A halves of the last
  chunk's projections (held psum accumulation), keeping the HAM clock
  gate warm through the tail.
"""

import sys
from collections import deque

import numpy as np

sys.path.insert(0, "/opt/trn_rl_repo")

import concourse.bass as bass  # noqa: E402
import concourse.bacc as bacc  # noqa: E402
import concourse.tile as tile  # noqa: E402
import concourse.mybir as mybir  # noqa: E402

F32 = mybir.dt.float32
BF16 = mybir.dt.bfloat16
ActFn = mybir.ActivationFunctionType

P = 128          # partition dim
CHUNK = 512      # i-chunk (matmul moving free dim, one psum bank of fp32)
DH = 64          # head dim
HPC = 4          # heads per core
HS = HPC * DH    # 256 per-core inner slice
DHA = DH + 1     # augmented head dim (ones column for softmax sums)
INNER = 1024     # total inner dim (16 heads x 64)
N_CORES = 8
GROUPS = [[0, 1, 2, 3], [4, 5, 6, 7]]


def build_nc(seq=2048, dim=1024, n_cores=N_CORES, groups=GROUPS, compile=True):
    """Build the SPMD Bass graph (identical on all cores)."""
    nch = seq // CHUNK          # i-chunks
    jpc = CHUNK // P            # j-tiles per chunk (4)
    njt = seq // P              # j-tiles
    nk = dim // P               # feature k-tiles
    nko = INNER // P            # inner k-tiles for the output projection
    grp = len(groups[0])        # replica group size (4)

    nc = bacc.Bacc("TRN2", target_bir_lowering=False, debug=False,
                   enable_asserts=False, num_devices=n_cores)

    # all inputs are host-repacked partition-major: [P, ntiles*cols]
    xT = nc.dram_tensor("xT", [P, nk * seq], BF16, kind="ExternalInput").ap()
    wq = nc.dram_tensor("wq", [P, nk * HS], BF16, kind="ExternalInput").ap()
    wk = nc.dram_tensor("wk", [P, nk * HS], BF16, kind="ExternalInput").ap()
    wv = nc.dram_tensor("wv", [P, nk * HS], BF16, kind="ExternalInput").ap()
    wo = nc.dram_tensor("wo", [P, nko * HS], BF16, kind="ExternalInput").ap()
    mask_c = nc.dram_tensor("mask_c", [P, P], BF16, kind="ExternalInput").ap()
    outT = nc.dram_tensor("outT", [HS, seq], BF16, kind="ExternalOutput").ap()

    with tile.TileContext(nc) as tc:
        with tc.tile_pool(name="sb", bufs=1) as sb, \
             tc.tile_pool(name="ps", bufs=1, space="PSUM") as ps, \
             tc.tile_pool(name="dram", bufs=1, space="DRAM") as dram:

            dma_engines = [nc.sync, nc.scalar, nc.gpsimd]

            # warm up the collectives firmware first thing (tiny: the entry
            # barrier's length varies run to run; triggering before any
            # queued input DMAs keeps the CC stream clear for chunk 0's AG)
            warm_in = dram.tile([P, 4], BF16, tag="warm_i", name="warm_i")
            warm_out = dram.tile([grp * P, 4], BF16,
                                 tag="warm_o", name="warm_o")
            nc.sync.dma_start(warm_in[:], xT[0:P, 0:4])
            nc.gpsimd.collective_compute(
                "AllGather", mybir.AluOpType.bypass, replica_groups=groups,
                ins=[warm_in.opt()], outs=[warm_out.opt()])

            # ---- input loads: few, large, contiguous DMAs; xt halves
            # k-grouped across the three DMA queues so chunk-0 columns of
            # every k-tile land within the first few microseconds ----
            xts = sb.tile([P, nk * seq], BF16, tag="xts", name="xts")
            xt = [xts[:, k * seq:(k + 1) * seq] for k in range(nk)]
            wqs = sb.tile([P, nk * HS], BF16, tag="wqs", name="wqs")
            wq_sb = [wqs[:, k * HS:(k + 1) * HS] for k in range(nk)]
            wks = sb.tile([P, nk * HS], BF16, tag="wks", name="wks")
            wk_sb = [wks[:, k * HS:(k + 1) * HS] for k in range(nk)]
            wvs = sb.tile([P, nk * HS], BF16, tag="wvs", name="wvs")
            wv_sb = [wvs[:, k * HS:(k + 1) * HS] for k in range(nk)]
            wos = sb.tile([P, nko * HS], BF16, tag="wos", name="wos")
            wo_sb = [wos[:, k * HS:(k + 1) * HS] for k in range(nko)]
            mask_sb = sb.tile([P, P], BF16, tag="mask", name="mask")

            def kslice(t, k0, k1, cols):
                return t.rearrange("p (n c) -> p n c", n=nk)[:, k0:k1, cols]

            q0 = slice(0, min(CHUNK, seq))
            q1 = slice(min(CHUNK, seq), min(2 * CHUNK, seq))
            h1 = slice(min(2 * CHUNK, seq), seq)
            kb = max(1, (nk + 2) // 3)
            kg = [(a, min(a + kb, nk)) for a in range(0, nk, kb)]
            wh = nk * HS // 2
            nc.gpsimd.dma_start(mask_sb[:], mask_c[:])
            nc.sync.dma_start(wqs[:, 0:wh], wq[:, 0:wh])
            nc.scalar.dma_start(wqs[:, wh:], wq[:, wh:])
            for cols in (q0, q1, h1):
                if cols.start >= cols.stop:
                    continue
                for (k0, k1), eng in zip(kg, dma_engines):
                    eng.dma_start(kslice(xts, k0, k1, cols),
                                  kslice(xT, k0, k1, cols))
                if cols is q0:
                    nc.sync.dma_start(wks[:, 0:wh], wk[:, 0:wh])
                    nc.scalar.dma_start(wks[:, wh:], wk[:, wh:])
                    nc.gpsimd.dma_start(wvs[:], wv[:])
            nc.scalar.dma_start(wos[:], wo[:])

            # persistent QKV results
            qt_sb = [sb.tile([P, seq], BF16, tag=f"qt{p}", name=f"qt{p}")
                     for p in range(2)]
            kt_sb = [sb.tile([P, seq], BF16, tag=f"kt{p}", name=f"kt{p}")
                     for p in range(2)]
            v_sb = [sb.tile([P, HPC * DHA], BF16, tag=f"v{j}", name=f"v{j}")
                    for j in range(njt)]
            ot_sb = [sb.tile([P, seq], BF16, tag=f"ot{p}", name=f"ot{p}")
                     for p in range(2)]

            # ---- interleavable work items (each emits one psum group) ----
            def emit_kt(pair, ch):
                pt = ps.tile([P, CHUNK], F32, tag="misc",
                             name=f"ktps{pair}_{ch}", bufs=2)
                for k in range(nk):
                    nc.tensor.matmul(
                        pt[:], lhsT=wk_sb[k][:, pair * P:(pair + 1) * P],
                        rhs=xt[k][:, ch * CHUNK:(ch + 1) * CHUNK],
                        start=(k == 0), stop=(k == nk - 1))
                nc.vector.tensor_copy(
                    kt_sb[pair][:, ch * CHUNK:(ch + 1) * CHUNK], pt[:])

            def emit_v(jt):
                pt = ps.tile([P, HS], F32, tag="misc",
                             name=f"vps{jt}", bufs=2)
                for k in range(nk):
                    nc.tensor.matmul(
                        pt[:], lhsT=xt[k][:, jt * P:(jt + 1) * P],
                        rhs=wv_sb[k][:],
                        start=(k == 0), stop=(k == nk - 1))
                nc.vector.tensor_copy(
                    v_sb[jt].rearrange("p (h d) -> p h d", h=HPC)[:, :, 0:DH],
                    pt.rearrange("p (h d) -> p h d", h=HPC))
                nc.vector.memset(
                    v_sb[jt].rearrange("p (h d) -> p h d", h=HPC)[:, :, DH:DHA],
                    1.0)

            def emit_qt(pair, ch):
                pt = ps.tile([P, CHUNK], F32, tag="misc",
                             name=f"qps{pair}_{ch}", bufs=2)
                for k in range(nk):
                    nc.tensor.matmul(
                        pt[:],
                        lhsT=wq_sb[k][:, pair * P:(pair + 1) * P],
                        rhs=xt[k][:, ch * CHUNK:(ch + 1) * CHUNK],
                        start=(k == 0), stop=(k == nk - 1))
                nc.vector.tensor_copy(
                    qt_sb[pair][:, ch * CHUNK:(ch + 1) * CHUNK], pt[:])

            def emit_proj(ci, m, slices, op_ps=None, evac=True):
                # transposed output block: outT[m*128:(m+1)*128, chunk ci]
                # = Wo[:, m-slice].T @ attT[:, chunk] over the k-tiles in
                # `slices` (a partial pass keeps op_ps alive)
                c0 = ci * CHUNK
                first = op_ps is None
                if first:
                    op_ps = ps.tile([P, CHUNK], F32, tag="misc",
                                    name=f"op{ci}_{m}", bufs=2)
                for n, (k, ag_t, coff) in enumerate(slices):
                    nc.tensor.matmul(
                        op_ps[:],
                        lhsT=wo_sb[k][:, m * P:(m + 1) * P],
                        rhs=ag_t[:, coff:coff + CHUNK],
                        start=(first and n == 0),
                        stop=(evac and n == len(slices) - 1))
                if not evac:
                    return op_ps
                o_sb = sb.tile([P, CHUNK], BF16, tag="osb",
                               name=f"o{ci}_{m}", bufs=4)
                nc.vector.tensor_copy(o_sb[:], op_ps[:])
                nc.sync.dma_start(outT[m * P:(m + 1) * P, c0:c0 + CHUNK],
                                  o_sb[:])
                return None

            work_early = deque()   # KT/V/Q for future chunks (not gated)
            work_late = deque()    # output projections (gated on AllGather)

            def pop_work(late_ok, late_floor):
                if work_early:
                    work_early.popleft()()
                    if len(work_early) > 4:
                        work_early.popleft()()
                elif late_ok and len(work_late) > late_floor:
                    work_late.popleft()()

            def emit_ag_full(ci, bounce_in):
                # one AllGather for both head pairs of chunk `ci` (256KB —
                # amortizes the ncfw floor; rank-major rows land so that
                # gathered row-block k*128 is exactly attT k-tile k)
                bounce_out = dram.tile([grp * 2 * P, CHUNK], BF16,
                                       tag="boutf", name=f"boutf{ci}", bufs=2)
                nc.gpsimd.collective_compute(
                    "AllGather", mybir.AluOpType.bypass,
                    replica_groups=groups,
                    ins=[bounce_in.opt()], outs=[bounce_out.opt()])
                tiles = {}
                for k in range(nko):
                    # gated loads stay on sync: on scalar/gpsimd they would
                    # head-of-line-block exp/broadcast until the AG lands
                    t = sb.tile([P, CHUNK], BF16, tag=f"ag{k}",
                                name=f"ag{ci}_{k}", bufs=3)
                    nc.sync.dma_start(t[:], bounce_out[k * P:(k + 1) * P, :])
                    tiles[k] = t
                return tiles

            def emit_ag_pair(ci, pair):
                # half AllGather (one head pair) of the chunk `ci` — fired
                # right after that pair's normalize, so pair A overlaps the
                # second attention pass and both stay small (cheap on CC).
                c0 = ci * CHUNK
                bounce_in = dram.tile([P, CHUNK], BF16, tag=f"binh{pair}",
                                      name=f"binh{ci}_{pair}", bufs=2)
                bounce_out = dram.tile([grp * P, CHUNK], BF16,
                                       tag=f"bouth{pair}",
                                       name=f"bouth{ci}_{pair}", bufs=2)
                nc.sync.dma_start(bounce_in[:], ot_sb[pair][:, c0:c0 + CHUNK])
                nc.gpsimd.collective_compute(
                    "AllGather", mybir.AluOpType.bypass,
                    replica_groups=groups,
                    ins=[bounce_in.opt()], outs=[bounce_out.opt()])
                tiles = {}
                for r in range(grp):
                    k = 2 * r + pair
                    t = sb.tile([P, CHUNK], BF16, tag=f"ag{k}",
                                name=f"ag{ci}_{k}", bufs=3)
                    # pair B fires after the last exp/broadcast, so its
                    # gated loads can spread over all three DMA queues;
                    # pair A's must not block scalar/gpsimd mid-pass-B
                    eng = dma_engines[r % 3] if pair == 1 else nc.sync
                    eng.dma_start(t[:], bounce_out[r * P:(r + 1) * P, :])
                    tiles[k] = t
                return tiles

            # ---- upfront projections: chunk-0/1 Q, chunk-0 K, chunk-0 V
            # (Q/K/V for later chunks are deferred into the work queue).
            for pair in range(2):
                emit_qt(pair, 0)
            for pair in range(2):
                emit_kt(pair, 0)
            for jt in range(jpc):
                emit_v(jt)
            if nch > 1:
                for pair in range(2):
                    emit_qt(pair, 1)
            for ch in range(2, nch):
                for pair in range(2):
                    work_early.append(
                        lambda pair=pair, ch=ch: emit_qt(pair, ch))

            # ---- attention chunks ----
            last_parts = {}
            for ci in range(nch):
                jt_end = jpc * (ci + 1)
                c0 = ci * CHUNK
                last = ci == nch - 1

                if ci + 1 < nch:
                    for pair in range(2):
                        work_early.append(
                            lambda pair=pair, ch=ci + 1: emit_kt(pair, ch))
                    for jt in range(jpc * (ci + 1), jpc * (ci + 2)):
                        work_early.append(lambda jt=jt: emit_v(jt))

                binf = None if last else dram.tile(
                    [2 * P, CHUNK], BF16, tag="binf", name=f"binf{ci}",
                    bufs=2)

                for hpass in range(2):
                    # heads 2*hpass, 2*hpass+1  (== head pair `hpass`)
                    ot_ps = [ps.tile([DHA, CHUNK], F32, tag=f"ot{h2}",
                                     name=f"ot{ci}_{hpass}_{h2}", bufs=1)
                             for h2 in range(2)]
                    for jt in range(jt_end):
                        rel = max(0, (jt - jpc * ci)) * P
                        diag = jt >= jpc * ci

                        s2 = ps.tile([P, 2 * CHUNK], F32, tag="s2",
                                     name=f"s{ci}_{hpass}_{jt}", bufs=2)
                        es = sb.tile([P, 2 * CHUNK], BF16, tag="es",
                                     name=f"es{ci}_{hpass}_{jt}", bufs=3)

                        for h2 in range(2):
                            # S^T tile = K_h @ Q_h^T (row-tiled, K=64)
                            nc.tensor.matmul(
                                s2[:, h2 * CHUNK + rel:(h2 + 1) * CHUNK],
                                lhsT=kt_sb[hpass][h2 * DH:(h2 + 1) * DH,
                                                  jt * P:(jt + 1) * P],
                                rhs=qt_sb[hpass][h2 * DH:(h2 + 1) * DH,
                                                 c0 + rel:c0 + CHUNK],
                                start=True, stop=True,
                                tile_position=(h2 * DH, 0))
                        # one exp for both heads (both psum banks)
                        nc.scalar.activation(
                            es.rearrange("p (t c) -> p t c", t=2)[:, :, rel:],
                            s2.rearrange("p (t c) -> p t c", t=2)[:, :, rel:],
                            ActFn.Exp)
                        if diag:
                            # band mask on the diagonal block, both heads
                            nc.vector.tensor_mul(
                                es.rearrange("p (t c) -> p t c",
                                             t=2)[:, :, rel:rel + P],
                                es.rearrange("p (t c) -> p t c",
                                             t=2)[:, :, rel:rel + P],
                                mask_sb.rearrange(
                                    "p (o c) -> p o c",
                                    o=1).broadcast_to((P, 2, P)))
                        for h2 in range(2):
                            h = 2 * hpass + h2
                            # O^T(+sums) accumulation: V_aug^T @ expS^T
                            nc.tensor.matmul(
                                ot_ps[h2][:, rel:CHUNK],
                                lhsT=v_sb[jt][:, h * DHA:(h + 1) * DHA],
                                rhs=es[:, h2 * CHUNK + rel:(h2 + 1) * CHUNK],
                                start=(jt == 0), stop=(jt == jt_end - 1))
                        # reservoir: drain gated projections only when their
                        # AllGather is surely complete (pass B for the
                        # previous-previous chunk); keep 2 back for the tail
                        pop_work(late_ok=(hpass == 1 or last),
                                 late_floor=3 if hpass == 1 else 4)

                    # normalize: rcp of the sums row (both heads fused),
                    # partition-broadcast, then one psum-read mul per head
                    # writes the normalized O^T to SBUF.  The broadcast
                    # rides gpsimd except on the final pass, where a PE
                    # ones-matmul keeps gpsimd clear for the AG trigger.
                    sr2 = sb.tile([1, 2 * CHUNK], F32, tag="sr",
                                  name=f"sr{ci}_{hpass}", bufs=2)
                    rcp2 = sb.tile([1, 2 * CHUNK], F32, tag="rcp",
                                   name=f"rcp{ci}_{hpass}", bufs=2)
                    nc.scalar.activation(sr2[:, 0:CHUNK], ot_ps[0][DH:DHA, :],
                                         ActFn.Copy)
                    nc.vector.tensor_copy(sr2[:, CHUNK:2 * CHUNK],
                                          ot_ps[1][DH:DHA, :])
                    nc.vector.reciprocal_approx_fast(rcp2[:], sr2[:])
                    bc2 = sb.tile([DH, 2 * CHUNK], F32, tag="bc",
                                  name=f"bc{ci}_{hpass}", bufs=2)
                    nc.gpsimd.partition_broadcast(bc2[:], rcp2[:],
                                                  channels=DH)
                    for h2 in range(2):
                        nc.vector.tensor_mul(
                            ot_sb[hpass][h2 * DH:(h2 + 1) * DH,
                                         c0:c0 + CHUNK],
                            ot_ps[h2][0:DH, :],
                            bc2[:, h2 * CHUNK:(h2 + 1) * CHUNK])

                    if last:
                        # this pair's half of the chunk goes out now
                        last_parts.update(emit_ag_pair(ci, hpass))
                    else:
                        nc.sync.dma_start(
                            binf[hpass * P:(hpass + 1) * P, :],
                            ot_sb[hpass][:, c0:c0 + CHUNK])

                if last:
                    agt = dict(last_parts)
                    last_parts = {}
                else:
                    agt = emit_ag_full(ci, binf)
                evens = [(k, agt[k], 0) for k in range(0, nko, 2)]
                odds = [(k, agt[k], 0) for k in range(1, nko, 2)]
                nm = HS // P
                if not last:
                    for m in range(nm):
                        work_late.append(
                            lambda ci=ci, m=m, s=evens + odds:
                            emit_proj(ci, m, s))
                else:
                    # split each output block's projection: the even k-tiles
                    # (pair-A AllGather, long landed) fill the pair-B AG's
                    # flight window; psum held across the split (no other
                    # psum users remain at the tail)
                    op_tiles = {}

                    def proj_ev(ci, m):
                        op_tiles[m] = emit_proj(ci, m, evens, evac=False)

                    def proj_od(ci, m):
                        emit_proj(ci, m, odds, op_ps=op_tiles.pop(m))

                    for m in range(nm):
                        work_late.append(
                            lambda ci=ci, m=m: proj_ev(ci, m))
                    for m in range(nm):
                        work_late.append(
                            lambda ci=ci, m=m: proj_od(ci, m))

            # tail: reservoir drains now — the reserved projections and the
            # pair-A halves fill the final AllGather's flight window before
            # the gated pair-B halves run
            while work_early or work_late:
                pop_work(late_ok=True, late_floor=0)

    if compile:
        nc.compile()
    return nc


def _pack(a, p=P):
    # [n*p, c] -> [p, n*c] partition-major repack
    n = a.shape[0] // p
    return np.ascontiguousarray(
        a.reshape(n, p, a.shape[1]).transpose(1, 0, 2).reshape(p, -1))


def make_in_maps(x, Wq, Wk, Wv, Wo, n_cores=N_CORES):
    import ml_dtypes
    bf16 = ml_dtypes.bfloat16
    scale = np.float32(DH ** -0.5)
    # band mask for the diagonal j-tile of S^T [j,i]: keep j <= i
    mask_b = np.triu(np.ones((P, P), np.float32)).astype(bf16)
    in_maps = []
    for c in range(n_cores):
        b, r = divmod(c, 4)
        hs = r * HS
        in_maps.append({
            "xT": _pack(np.ascontiguousarray(x[b].T)).astype(bf16),
            "wq": _pack(Wq[:, hs:hs + HS] * scale).astype(bf16),
            "wk": _pack(Wk[:, hs:hs + HS]).astype(bf16),
            "wv": _pack(Wv[:, hs:hs + HS]).astype(bf16),
            "wo": _pack(Wo[:, hs:hs + HS]).astype(bf16),
            "mask_c": mask_b,
        })
    return in_maps


def assemble_out(results, B, seq, n_cores=N_CORES):
    out = np.empty((B, seq, INNER), np.float32)
    for c in range(n_cores):
        b, r = divmod(c, 4)
        out[b][:, r * HS:(r + 1) * HS] = results[c]["outT"].T.astype(np.float32)
    return out


_NC_CACHE = {}


def kernel(x, Wq, Wk, Wv, Wo):
    from concourse import bass_utils
    x = np.asarray(x, np.float32)
    B, seq, dim = x.shape
    key = (seq, dim)
    if key not in _NC_CACHE:
        _NC_CACHE[key] = build_nc(seq=seq, dim=dim)
    nc = _NC_CACHE[key]
    in_maps = make_in_maps(x, np.asarray(Wq, np.float32),
                           np.asarray(Wk, np.float32),
                           np.asarray(Wv, np.float32),
                           np.asarray(Wo, np.float32))
    res = bass_utils.run_bass_kernel_spmd(
        nc, in_maps, core_ids=list(range(N_CORES)))
    return assemble_out(res.results, B, seq)
